# revision 3
# baseline (speedup 1.0000x reference)
"""Trainium2 Bass kernel for Gaussian-KDE logsumexp (nn_GaussianKernel).

out[n] = logsumexp_m( -0.5*||(y_n - x_m)/bw||^2 - Z ),  Z = D/2*log(2pi) + D*log(bw) + log(M)

Factorization used on-device (per query row n, data col m):
    A[n,m] = (y_n . x_m)/bw^2  -  ||x_m||^2/(2 bw^2)
    out[n] = max_m A[n,m] + log(sum_m exp(A[n,m] - max)) - ||y_n||^2/(2 bw^2) - Z

Sharding: data-parallel over the 2048 query rows -> 8 cores x 256 rows,
each core holds the full x dataset (matmul K=D=128 on partitions).
"""

import sys
from math import log, pi

import numpy as np

sys.path.insert(0, "/opt/trn_rl_repo")

import concourse.bacc as bacc
import concourse.bass as bass
import concourse.mybir as mybir
import concourse.tile as tile
from concourse.bass_utils import run_bass_kernel_spmd

BW = 0.1
N_QUERY = 2048
N_DATA = 2048
DIM = 128
N_CORES = 8
SHARD = N_QUERY // N_CORES  # 256 query rows per core

NEG_HALF_INV_BW2 = -0.5 / (BW * BW)  # -50.0
Z_CONST = 0.5 * DIM * log(2.0 * pi) + DIM * log(BW) + log(float(N_DATA))

NT = 512  # matmul moving free dim (one PSUM bank of fp32)
N_TILES = N_DATA // NT  # 4
M_TILES = SHARD // 128  # 2

_CACHE = {}


def _build_nc():
    dt = mybir.dt.float32
    fx = mybir.ActivationFunctionType
    nc = bacc.Bacc("TRN2", target_bir_lowering=False, debug=False)

    # Inputs (pre-laid-out on host): yt = (y_shard/bw^2).T, xt = x.T, ynat = y_shard
    yt = nc.dram_tensor("yt", [DIM, SHARD], dt, kind="ExternalInput")
    xt = nc.dram_tensor("xt", [DIM, N_DATA], dt, kind="ExternalInput")
    ynat = nc.dram_tensor("ynat", [SHARD, DIM], dt, kind="ExternalInput")
    out = nc.dram_tensor("out", [SHARD, 1], dt, kind="ExternalOutput")

    with tile.TileContext(nc) as tc:
        with (
            tc.tile_pool(name="io", bufs=1) as io,
            tc.tile_pool(name="psum", bufs=8, space=bass.MemorySpace.PSUM) as psum,
            tc.tile_pool(name="work", bufs=3) as work,
            tc.tile_pool(name="small", bufs=2) as small,
        ):
            # ---- loads (chunked so compute can start early) ----
            xt_sb = io.tile([DIM, N_DATA], dt, tag="xt")
            for t in range(N_TILES):
                nc.sync.dma_start(xt_sb[:, t * NT:(t + 1) * NT],
                                  xt[:, t * NT:(t + 1) * NT])
            yt_sb = io.tile([DIM, SHARD], dt, tag="yt")
            nc.sync.dma_start(yt_sb[:], yt[:])
            ynat_sb = io.tile([128, DIM], dt, tag="yn0")
            ynat_sb2 = io.tile([128, DIM], dt, tag="yn1")
            ynat_tiles = [ynat_sb, ynat_sb2]
            for mt in range(M_TILES):
                nc.sync.dma_start(ynat_tiles[mt][:], ynat[mt * 128:(mt + 1) * 128, :])

            # ---- x^2 on gpsimd (keeps DVE/ACT free), chunked ----
            xsq_sb = io.tile([DIM, N_DATA], dt, tag="xsq")
            for t in range(N_TILES):
                nc.gpsimd.tensor_tensor(xsq_sb[:, t * NT:(t + 1) * NT],
                                        xt_sb[:, t * NT:(t + 1) * NT],
                                        xt_sb[:, t * NT:(t + 1) * NT],
                                        op=mybir.AluOpType.mult)

            # constant matrix for the column-bias matmul: all -1/(2 bw^2)
            cmat = io.tile([DIM, 128], dt, tag="cmat")
            nc.vector.memset(cmat[:], NEG_HALF_INV_BW2)

            for mt in range(M_TILES):
                # ---- PE: A = cmat.T @ xsq  +  yt.T @ xt  (per 512-col bank) ----
                ps = [psum.tile([128, NT], dt, tag="ps", name=f"ps_{mt}_{t}")
                      for t in range(N_TILES)]
                for t in range(N_TILES):
                    nc.tensor.matmul(ps[t][:], cmat[:],
                                     xsq_sb[:, t * NT:(t + 1) * NT],
                                     start=True, stop=False)
                for t in range(N_TILES):
                    nc.tensor.matmul(ps[t][:], yt_sb[:, mt * 128:(mt + 1) * 128],
                                     xt_sb[:, t * NT:(t + 1) * NT],
                                     start=False, stop=True)

                # ---- DVE: row max over the 2048 cols (per-bank partials) ----
                pmax = small.tile([128, N_TILES], dt, tag="pmax")
                for t in range(N_TILES):
                    nc.vector.tensor_reduce(pmax[:, t:t + 1], ps[t][:],
                                            axis=mybir.AxisListType.X,
                                            op=mybir.AluOpType.max)
                nmax = small.tile([128, 1], dt, tag="nmax")
                nc.vector.tensor_reduce(nmax[:], pmax[:],
                                        axis=mybir.AxisListType.X,
                                        op=mybir.AluOpType.max, negate=True)

                # ---- ACT: exp(A - max) with fused row-sum accumulation ----
                esum = small.tile([128, N_TILES], dt, tag="esum")
                for t in range(N_TILES):
                    esc = work.tile([128, NT], dt, tag="esc")
                    nc.scalar.activation(esc[:], ps[t][:], fx.Exp,
                                         bias=nmax[:], scale=1.0,
                                         accum_out=esum[:, t:t + 1])

                tot = small.tile([128, 1], dt, tag="tot")
                nc.vector.tensor_reduce(tot[:], esum[:],
                                        axis=mybir.AxisListType.X,
                                        op=mybir.AluOpType.add)

                # ---- ||y_n||^2 (gpsimd square, DVE row-sum) ----
                ysq = small.tile([128, DIM], dt, tag="ysq")
                nc.gpsimd.tensor_tensor(ysq[:], ynat_tiles[mt][:], ynat_tiles[mt][:],
                                        op=mybir.AluOpType.mult)
                yn2 = small.tile([128, 1], dt, tag="yn2")
                nc.vector.tensor_reduce(yn2[:], ysq[:],
                                        axis=mybir.AxisListType.X,
                                        op=mybir.AluOpType.add)

                # ---- combine: out = -nmax + ln(tot) + yn2*(-1/(2bw^2)) - Z ----
                lnt = small.tile([128, 1], dt, tag="lnt")
                nc.scalar.activation(lnt[:], tot[:], fx.Ln)
                t1 = small.tile([128, 1], dt, tag="t1")
                nc.vector.tensor_sub(t1[:], lnt[:], nmax[:])
                t2 = small.tile([128, 1], dt, tag="t2")
                nc.vector.tensor_scalar(t2[:], yn2[:], NEG_HALF_INV_BW2, -Z_CONST,
                                        op0=mybir.AluOpType.mult,
                                        op1=mybir.AluOpType.add)
                osb = small.tile([128, 1], dt, tag="osb")
                nc.vector.tensor_add(osb[:], t1[:], t2[:])

                nc.sync.dma_start(out[mt * 128:(mt + 1) * 128, :], osb[:])

    nc.compile()
    return nc


def kernel(y, x):
    y = np.asarray(y, dtype=np.float32)
    x = np.asarray(x, dtype=np.float32)
    assert y.shape == (N_QUERY, DIM) and x.shape == (N_DATA, DIM)

    if "nc" not in _CACHE:
        _CACHE["nc"] = _build_nc()
    nc = _CACHE["nc"]

    xt = np.ascontiguousarray(x.T)
    in_maps = []
    for i in range(N_CORES):
        ysh = y[i * SHARD:(i + 1) * SHARD]
        in_maps.append({
            "yt": np.ascontiguousarray(ysh.T) * np.float32(1.0 / (BW * BW)),
            "ynat": np.ascontiguousarray(ysh),
            "xt": xt,
        })

    res = run_bass_kernel_spmd(nc, in_maps, core_ids=list(range(N_CORES)))
    return np.concatenate([r["out"].reshape(-1) for r in res.results]).astype(np.float32)
